# revision 2
# baseline (speedup 1.0000x reference)
import sys
sys.path.insert(0, '/opt/trn_rl_repo')
import numpy as np
import ml_dtypes
import concourse.bass as bass
import concourse.bacc as bacc
import concourse.tile as tile
from concourse import mybir
from concourse.bass_utils import run_bass_kernel_spmd

F32 = mybir.dt.float32
BF16 = mybir.dt.float16
AF = mybir.ActivationFunctionType
ALU = mybir.AluOpType
BF = np.float16


def _register_sigmul():
    """out = (in0*s0 + s1) * in1 as one DVE instruction (sigmoid-from-tanh
    folded into the following multiply). Registered at import; idempotent."""
    from concourse.dve_ops import (DveOp, OPS, CUSTOM_DVE_SPECS,
                                   _SUB_OPCODE_FOR_NAME, _CUSTOM_DVE_ROW_BASE)
    from concourse.dve_spec import Spec, Src0, Src1, C0, C1, lower
    from concourse.dve_uop import DveOpSpec
    if 'SIGMUL_ANT' in _SUB_OPCODE_FOR_NAME:
        return next(op for op in OPS if op.name == 'SIGMUL_ANT')
    spec = Spec(
        body=(Src0 * C0 + C1) * Src1,
        reference=lambda in0, in1, s0, s1, imm2:
            (in0.astype(np.float32) * s0 + s1) * in1,
    )
    opcode = _CUSTOM_DVE_ROW_BASE + len(OPS)
    shas = {}
    for ver in ("v3", "v4"):
        uops = lower(spec, ver=ver)
        shas[ver] = DveOpSpec(name='SIGMUL_ANT', opcode=opcode, uops=uops,
                              rd1_en=True).sha(ver)
    op = DveOp('SIGMUL_ANT', spec, subdim=False, uops_sha=shas)
    OPS.append(op)
    _SUB_OPCODE_FOR_NAME[op.name] = opcode
    CUSTOM_DVE_SPECS[op.name] = spec
    return op


SIGMUL = _register_sigmul()

U = 400        # LSTM units
KATT = 10      # attention gaussians
NCHARS = 73    # alphabet
NMIX = 20      # GMM components
UC = 50        # char positions
NB = 4         # batch per core
NCORES = 8
NOUT = 6 * NMIX + 1  # 121

# m-tiles: per gate [128,128,128,16] -> 16 m-tiles, psum z [128, 64]
MW = [128, 128, 128, 16] * 4
SOFF = [400 * (m // 4) + 128 * (m % 4) for m in range(16)]   # weight col offsets
XWROWS = 100  # xw tile: w @0:73, junk, x @96:99, bias(1.0) @99

_CACHE = {}


def _pack_wblocks(W, nkt):
    """W [rows<=128*nkt, 1600 cols] -> [128, nkt*1600] k-major, bf16."""
    out = np.zeros((128, nkt * 1600), BF)
    r = W.shape[0]
    for k in range(nkt):
        lo, hi = 128 * k, min(128 * (k + 1), r)
        if lo >= r:
            break
        out[0: hi - lo, k * 1600: k * 1600 + 1600] = W[lo:hi].astype(BF)
    return out


def _build_program(T):
    nc = bacc.Bacc("TRN2", target_bir_lowering=False, debug=False,
                   num_devices=NCORES)

    dW1 = nc.dram_tensor("W1", [128, 5 * 1600], BF16, kind="ExternalInput").ap()
    dW2 = nc.dram_tensor("W2", [128, 9 * 1600], BF16, kind="ExternalInput").ap()
    dW3 = nc.dram_tensor("W3", [128, 9 * 1600], BF16, kind="ExternalInput").ap()
    dPB = nc.dram_tensor("PB", [128, 9 * 16], F32, kind="ExternalInput").ap()
    dXT = nc.dram_tensor("XT", [3, (T + 1) * NB], BF16, kind="ExternalInput").ap()
    dWATT = nc.dram_tensor("WATT", [128, 4 * 30], BF16, kind="ExternalInput").ap()
    dWATTB = nc.dram_tensor("WATTB", [1, 30], BF16, kind="ExternalInput").ap()
    dONES = nc.dram_tensor("ONESB", [1, NB], BF16, kind="ExternalInput").ap()
    dV3 = nc.dram_tensor("V3", [1, 3 * UC], F32, kind="ExternalInput").ap()
    dOHB = nc.dram_tensor("OHB", [UC, NB * NCHARS], BF16, kind="ExternalInput").ap()
    dWMDN = nc.dram_tensor("WMDN", [128, 5 * 200], BF16, kind="ExternalInput").ap()
    dONESC = nc.dram_tensor("ONESC", [1, 400], BF16, kind="ExternalInput").ap()
    dXWI = nc.dram_tensor("XWI", [XWROWS, NB], BF16, kind="ExternalInput").ap()
    dZB = nc.dram_tensor("ZB", [128, 16], BF16, kind="ExternalInput").ap()
    dOUT1 = nc.dram_tensor("OUT1", [128, T * NB], F32, kind="ExternalOutput").ap()
    dOUT2 = nc.dram_tensor("OUT2", [72, T * NB], F32, kind="ExternalOutput").ap()
    import os as _os
    _DBG = bool(_os.environ.get("KDBG"))
    if _DBG:
        dDH1 = nc.dram_tensor("DH1", [128, 16], BF16, kind="ExternalOutput").ap()
        dDH2 = nc.dram_tensor("DH2", [128, 16], BF16, kind="ExternalOutput").ap()
        dDH3 = nc.dram_tensor("DH3", [128, 16], BF16, kind="ExternalOutput").ap()
        dDC1 = nc.dram_tensor("DC1", [128, 32], F32, kind="ExternalOutput").ap()
        dDKAP = nc.dram_tensor("DKAP", [1, 40], F32, kind="ExternalOutput").ap()
        dDXW = nc.dram_tensor("DXW", [XWROWS, NB], BF16, kind="ExternalOutput").ap()
        dDZ1 = nc.dram_tensor("DZ1", [128, 64], F32, kind="ExternalOutput").ap()
        dDE = nc.dram_tensor("DE", [1, 120], F32, kind="ExternalOutput").ap()
        dDPHI = nc.dram_tensor("DPHI", [UC, NB], F32, kind="ExternalOutput").ap()

    with tile.TileContext(nc) as tc:
        with tc.tile_pool(name="statics", bufs=1) as statics, \
             tc.tile_pool(name="states", bufs=1) as states:

            sW1 = statics.tile([128, 5 * 1600], BF16)
            sW2 = statics.tile([128, 9 * 1600], BF16)
            sW3 = statics.tile([128, 9 * 1600], BF16)
            sPB = statics.tile([128, 9 * 16], F32)
            sXT = statics.tile([3, (T + 1) * NB], BF16)
            sWATT = statics.tile([128, 4 * 30], BF16)
            sWATTB = statics.tile([1, 30], BF16)
            sONES = statics.tile([1, NB], BF16)
            sV3 = statics.tile([1, 3 * UC], F32)
            sOHB = statics.tile([UC, NB * NCHARS], BF16)
            sWMDN = statics.tile([128, 5 * 200], BF16)
            sONESC = statics.tile([1, 400], BF16)
            for dst, src in [(sW1, dW1), (sW2, dW2), (sW3, dW3), (sPB, dPB),
                             (sXT, dXT), (sWATT, dWATT), (sWATTB, dWATTB),
                             (sONES, dONES), (sV3, dV3), (sOHB, dOHB),
                             (sWMDN, dWMDN), (sONESC, dONESC)]:
                nc.gpsimd.dma_start(out=dst[:], in_=src[:])

            h3all = states.tile([128, (T + 1) * 16], BF16)
            OUTS1 = states.tile([128, T * NB], F32)
            OUTS2 = states.tile([72, T * NB], F32)
            h = [states.tile([128, 16], BF16, name=f"h{i}") for i in range(3)]
            # CZO[l]: cols 0:16 = zo_ scratch, cols 16:32 = c state (f32)
            czo = [states.tile([128, 32], F32, name=f"czo{i}") for i in range(3)]
            xwA = states.tile([XWROWS, NB], BF16)
            # part2 carries across the rotated loop boundary
            part2c = states.tile([128, 64], F32)
            kap = states.tile([1, NB * KATT], F32)  # (b, k)
            for tl in h:
                nc.gpsimd.dma_start(out=tl[:], in_=dZB[:])
            nc.gpsimd.dma_start(out=xwA[:], in_=dXWI[:])
            nc.vector.memset(part2c[:], 0.0)
            for tl in czo:
                nc.vector.memset(tl[:], 0.0)
            nc.vector.memset(kap[:], 0.0)

            with tc.tile_pool(name="psum", bufs=1, space="PSUM") as psum, \
                 tc.tile_pool(name="scratch", bufs=2) as scratch:

                # zA: shared accumulator for L1 + L2/L3 xw groups
                zA = psum.tile([128, 64], F32, name="zA")
                # split accumulators: self and skip k-tiles as closed groups
                zS = [psum.tile([128, 64], F32, name=f"zS{i}") for i in range(2)]
                zK = [psum.tile([128, 64], F32, name=f"zK{i}") for i in range(2)]
                attp = psum.tile([1, NB * 30], F32)
                # argp and wp share one bank (their groups never overlap)
                argwp = psum.tile([NCHARS, NB * KATT + NB], F32)
                argp = argwp[0:UC, 0:NB * KATT]
                wp = argwp[0:NCHARS, NB * KATT:NB * KATT + NB]
                for tl in [zA] + zS + zK:
                    nc.vector.memset(tl[:], 0.0)

                _sc = {}
                for l in range(3):
                    for _t, _shp in [(f"pci{l}", [128, 16]), (f"pcf{l}", [128, 16]),
                                     (f"pco{l}", [128, 16]),
                                     (f"zif{l}", [128, 32]), (f"tif{l}", [128, 32]),
                                     (f"tg{l}", [128, 16]), (f"toc{l}", [128, 32]),
                                     (f"fg{l}", [128, 16]),
                                     (f"m1{l}", [128, 16]), (f"m2{l}", [128, 16])]:
                        _sc[_t] = scratch.tile(_shp, F32, tag=_t, name=_t)
                for _t, _shp, _dt in [("zsum1", [128, 64], F32),
                                      ("zsum2", [128, 64], F32),
                                      ("part1", [128, 64], F32),
                                      ("part2", [128, 64], F32),
                                      ("sZS1", [128, 64], F32),
                                      ("sZS2", [128, 64], F32),
                                      ("E", [1, NB * 30], F32),
                                      ("bk", [1, NB * KATT], F32),
                                      ("A_", [1, NB * KATT], F32),
                                      ("bk2", [1, NB * KATT], F32),
                                      ("B_", [1, NB * KATT], F32),
                                      ("C_", [1, NB * KATT], F32),
                                      ("P", [UC, NB * KATT], F32),
                                      ("phib", [UC, NB], BF16)]:
                    _sc[_t] = scratch.tile(_shp, _dt, tag=_t, name=_t)

                def layer_mms(dst, sW, movs):
                    n = len(movs)
                    for m in range(16):
                        w0, mw = int(SOFF[m]), MW[m]
                        for ki, (mov, kidx) in enumerate(movs):
                            kp = mov.shape[0]
                            nc.tensor.matmul(
                                dst[0:mw, 4 * m: 4 * m + 4],
                                sW[0:kp, kidx * 1600 + w0: kidx * 1600 + w0 + mw],
                                mov,
                                start=(ki == 0),
                                stop=(ki == n - 1),
                            )

                def st(tag):
                    return _sc[tag]

                def peep_prefetch(l):
                    # p_i*c and p_f*c only need c(t-1): run during the MM drain
                    p0 = sPB[:, (3 * l + 0) * 16: (3 * l + 0) * 16 + 16]
                    p1 = sPB[:, (3 * l + 1) * 16: (3 * l + 1) * 16 + 16]
                    c = czo[l][:, 16:32]
                    nc.gpsimd.tensor_tensor(out=st(f"pci{l}")[:], in0=p0, in1=c,
                                            op=ALU.mult)
                    nc.gpsimd.tensor_tensor(out=st(f"pcf{l}")[:], in0=p1, in1=c,
                                            op=ALU.mult)

                def cell(l, t, zt):
                    zi, zf = zt[:, 0:16], zt[:, 16:32]
                    zg, zo = zt[:, 32:48], zt[:, 48:64]
                    p2 = sPB[:, (3 * l + 2) * 16: (3 * l + 2) * 16 + 16]
                    c = czo[l][:, 16:32]
                    zif = st(f"zif{l}")
                    tif = st(f"tif{l}")
                    toc = st(f"toc{l}")
                    pci, pcf, pco = st(f"pci{l}"), st(f"pcf{l}"), st(f"pco{l}")
                    fg = st(f"fg{l}")
                    m1, m2 = st(f"m1{l}"), st(f"m2{l}")
                    tg = st(f"tg{l}")
                    # input/forget gates (sigmoid via tanh): one ACT over [zi_|zf_]
                    nc.vector.tensor_tensor(out=zif[:, 0:16], in0=zi, in1=pci[:],
                                            op=ALU.add)
                    nc.vector.tensor_tensor(out=zif[:, 16:32], in0=zf, in1=pcf[:],
                                            op=ALU.add)
                    nc.scalar.activation(out=tif[:], in_=zif[:], func=AF.Tanh,
                                         scale=0.5)
                    nc.scalar.activation(out=tg[:], in_=zg, func=AF.Tanh)
                    # m1 = sigmoid_i * tanh(zg), fused (0.5*ti+0.5)*tg
                    nc.vector._custom_dve(SIGMUL, out=m1[:], in0=tif[:, 0:16],
                                          in1=tg[:], s0=0.5, s1=0.5, imm2=0.0)
                    # m2 = sigmoid_f * c_old on gpsimd (classic two ops)
                    nc.gpsimd.tensor_scalar(out=fg[:], in0=tif[:, 16:32], scalar1=0.5,
                                            scalar2=0.5, op0=ALU.mult, op1=ALU.add)
                    nc.gpsimd.tensor_tensor(out=m2[:], in0=fg[:], in1=c, op=ALU.mult)
                    # c_new -> czo[l][:,16:32]
                    nc.vector.tensor_tensor(out=c, in0=m1[:], in1=m2[:], op=ALU.add)
                    nc.vector.tensor_tensor(out=pco[:], in0=p2, in1=c, op=ALU.mult)
                    nc.vector.tensor_tensor(out=czo[l][:, 0:16], in0=zo, in1=pco[:],
                                            op=ALU.add)
                    # one ACT over [zo_|c] -> [to|tanh(c)]; the o-gate weights
                    # and p_o are prescaled by 0.5 host-side, so scale=1.0
                    nc.scalar.activation(out=toc[:], in_=czo[l][:], func=AF.Tanh)
                    # h = sigmoid_o * tanh(c), fused; writes fp16
                    nc.vector._custom_dve(SIGMUL, out=h[l][:], in0=toc[:, 0:16],
                                          in1=toc[:, 16:32], s0=0.5, s1=0.5,
                                          imm2=0.0)
                    if l == 2:
                        nc.gpsimd.tensor_copy(
                            out=h3all[:, bass.DynSlice(t, 16)], in_=h[2][:])

                def attention_mms():
                    for b in range(NB):
                        for k in range(4):
                            nc.tensor.matmul(
                                attp[0:1, 30 * b: 30 * b + 30],
                                h[0][:, 4 * k + b: 4 * k + b + 1],
                                sWATT[:, 30 * k: 30 * k + 30],
                                start=(k == 0), stop=False,
                            )
                        nc.tensor.matmul(
                            attp[0:1, 30 * b: 30 * b + 30],
                            sONES[0:1, b: b + 1],
                            sWATTB[0:1, :],
                            start=False, stop=True,
                        )

                def attention_vec1():
                    E = _sc["E"]
                    nc.scalar.activation(out=E[:], in_=attp[:], func=AF.Exp)
                    Ev = E[0:1, :].rearrange("p (b x) -> p b x", b=NB)
                    av = attp[0:1, :].rearrange("p (b x) -> p b x", b=NB)
                    kapv = kap[0:1, :].rearrange("p (b k) -> p b k", b=NB)
                    # kappa += exp(k_hat), in place
                    nc.vector.tensor_tensor(out=kapv, in0=kapv, in1=Ev[:, :, 20:30],
                                            op=ALU.add)
                    bk = _sc["bk"]
                    bkv = bk[0:1, :].rearrange("p (b k) -> p b k", b=NB)
                    nc.gpsimd.tensor_tensor(out=bkv, in0=Ev[:, :, 10:20], in1=kapv,
                                            op=ALU.mult)
                    A_ = _sc["A_"]
                    Av = A_[0:1, :].rearrange("p (b k) -> p b k", b=NB)
                    bk2 = _sc["bk2"]
                    bk2v = bk2[0:1, :].rearrange("p (b k) -> p b k", b=NB)
                    nc.gpsimd.tensor_tensor(out=bk2v, in0=bkv, in1=kapv, op=ALU.mult)
                    nc.vector.tensor_tensor(out=Av, in0=av[:, :, 0:10], in1=bk2v,
                                            op=ALU.subtract)
                    B_ = _sc["B_"]
                    nc.gpsimd.tensor_scalar(out=B_[:], in0=bk[:], scalar1=2.0,
                                            scalar2=None, op0=ALU.mult)
                    C_ = _sc["C_"]
                    Cv = C_[0:1, :].rearrange("p (b k) -> p b k", b=NB)
                    nc.gpsimd.tensor_scalar(out=Cv, in0=Ev[:, :, 10:20], scalar1=-1.0,
                                            scalar2=None, op0=ALU.mult)

                def attention_arg_mms():
                    # arg[u,(b,k)] = A + u*B + u^2*C via 3 accumulating K=1 matmuls
                    nc.tensor.matmul(argp[:], sV3[0:1, 0:UC], _sc["A_"][:],
                                     start=True, stop=False)
                    nc.tensor.matmul(argp[:], sV3[0:1, UC:2 * UC], _sc["B_"][:],
                                     start=False, stop=False)
                    nc.tensor.matmul(argp[:], sV3[0:1, 2 * UC:3 * UC], _sc["C_"][:],
                                     start=False, stop=True)

                def attention_vec2():
                    P = _sc["P"]
                    nc.scalar.activation(out=P[:], in_=argp[:], func=AF.Exp)
                    phib = _sc["phib"]
                    Pv = P[:, :].rearrange("p (b k) -> p b k", b=NB)
                    with nc.allow_low_precision("phi values are O(10); fp16 ok"):
                        nc.vector.tensor_reduce(out=phib[:], in_=Pv,
                                                axis=mybir.AxisListType.X,
                                                op=ALU.add)

                def attention_w_mms():
                    phib = _sc["phib"]
                    for b in range(NB):
                        nc.tensor.matmul(
                            wp[:, b: b + 1],
                            sOHB[:, NCHARS * b: NCHARS * b + NCHARS],
                            phib[:, b: b + 1],
                            start=True, stop=True,
                        )

                def h_movs(tl):
                    return [tl[:, 0:4], tl[:, 4:8], tl[:, 8:12], tl[:, 12:16]]

                sZS1, sZS2 = _sc["sZS1"], _sc["sZS2"]
                part1 = _sc["part1"]
                zsum1, zsum2 = _sc["zsum1"], _sc["zsum2"]

                import os as _os2
                _sres = not _os2.environ.get("KNOSTAG")
                with tc.For_i(0, T, staggered_reset=_sres) as t:
                    # peephole products (need only c of t-1): run on gpsimd
                    # while the L1 matmuls drain
                    for l in range(3):
                        peep_prefetch(l)
                    layer_mms(zA, sW1, list(zip(h_movs(h[0]), range(4)))
                              + [(xwA[:], 4)])
                    cell(0, t * 16 + 16, zA)
                    # self groups (fill cell0 latency)
                    layer_mms(zS[0], sW2, list(zip(h_movs(h[1]), range(4))))
                    layer_mms(zS[1], sW3, list(zip(h_movs(h[2]), range(4))))
                    nc.scalar.activation(out=sZS1[:], in_=zS[0][:], func=AF.Copy)
                    nc.scalar.activation(out=sZS2[:], in_=zS[1][:], func=AF.Copy)
                    attention_mms()
                    layer_mms(zK[0], sW2, list(zip(h_movs(h[0]), range(4, 8))))
                    attention_vec1()
                    attention_arg_mms()
                    attention_vec2()
                    nc.vector.tensor_tensor(out=part1[:], in0=sZS1[:],
                                            in1=zK[0][:], op=ALU.add)
                    attention_w_mms()
                    nc.vector.tensor_copy(out=xwA[0:73, :], in_=wp[:])
                    layer_mms(zA, sW2, [(xwA[:], 8)])
                    nc.vector.tensor_tensor(out=zsum1[:], in0=zA[:],
                                            in1=part1[:], op=ALU.add)
                    cell(1, t * 16 + 16, zsum1)
                    layer_mms(zK[1], sW3, list(zip(h_movs(h[1]), range(4, 8))))
                    nc.vector.tensor_tensor(out=part2c[:], in0=sZS2[:],
                                            in1=zK[1][:], op=ALU.add)
                    layer_mms(zA, sW3, [(xwA[:], 8)])
                    # prefetch x(t+1) so L1(t+1) can start during cell2
                    nc.vector.tensor_copy(out=xwA[96:99, :],
                                          in_=sXT[0:3, bass.DynSlice(t * NB + NB,
                                                                     NB)])
                    nc.vector.tensor_tensor(out=zsum2[:], in0=zA[:],
                                            in1=part2c[:], op=ALU.add)
                    cell(2, t * 16 + 16, zsum2)

                if _DBG:
                    dbgz = states.tile([128, 64], F32)
                    nc.vector.tensor_copy(out=dbgz[:], in_=zA[:])
                    dbgE = states.tile([1, 120], F32)
                    nc.vector.tensor_copy(out=dbgE[:], in_=_sc["E"][:])
                    dbgphi = states.tile([UC, NB], F32)
                    nc.vector.tensor_copy(out=dbgphi[:], in_=_sc["phib"][:])

            # ---- MDN head ----
            # Y1 rows: mu @0:40, eos @64, rho @96:116 ; Y2 rows: pi @0:20, s @32:72
            with tc.tile_pool(name="mpsum", bufs=2, space="PSUM") as mpsum, \
                 tc.tile_pool(name="mscr", bufs=2) as mscr, \
                 tc.tile_pool(name="mones", bufs=1) as mones:
                ones20 = mones.tile([NMIX, 1], F32)
                nc.vector.memset(ones20[:], 1.0)
                ones1_20 = mones.tile([1, NMIX], F32)
                nc.vector.memset(ones1_20[:], 1.0)
                h3v = h3all[:, 16:].rearrange("p (t x) -> p t x", t=T)
                CC = min(400, T * NB)
                TC = CC // NB
                for ch in range((T + TC - 1) // TC):
                    t0 = TC * ch
                    tn = min(TC, T - t0)
                    cc = tn * NB
                    yp1 = mpsum.tile([128, CC], F32, tag="yp1")
                    yp2 = mpsum.tile([72, CC], F32, tag="yp2")
                    for k in range(4):
                        nc.tensor.matmul(
                            yp1[0:128, 0:cc],
                            sWMDN[:, 200 * k: 200 * k + 128],
                            h3v[:, t0: t0 + tn, 4 * k: 4 * k + 4],
                            start=(k == 0), stop=False)
                    nc.tensor.matmul(
                        yp1[0:128, 0:cc],
                        sWMDN[0:1, 800: 800 + 128],
                        sONESC[0:1, 0:cc],
                        start=False, stop=True)
                    for k in range(4):
                        nc.tensor.matmul(
                            yp2[0:72, 0:cc],
                            sWMDN[:, 200 * k + 128: 200 * k + 200],
                            h3v[:, t0: t0 + tn, 4 * k: 4 * k + 4],
                            start=(k == 0), stop=False)
                    nc.tensor.matmul(
                        yp2[0:72, 0:cc],
                        sWMDN[0:1, 928: 928 + 72],
                        sONESC[0:1, 0:cc],
                        start=False, stop=True)
                    o1 = OUTS1[:, NB * t0: NB * t0 + cc]
                    o2 = OUTS2[:, NB * t0: NB * t0 + cc]
                    # pi softmax (pi lives at yp2[0:20])
                    epi = mscr.tile([NMIX, CC], F32, tag="epi")
                    nc.scalar.activation(out=epi[0:NMIX, 0:cc], in_=yp2[0:NMIX, 0:cc],
                                         func=AF.Exp)
                    sp = mpsum.tile([1, CC], F32, tag="sp")
                    nc.tensor.matmul(sp[0:1, 0:cc], ones20[:], epi[0:NMIX, 0:cc],
                                     start=True, stop=True)
                    rec = mscr.tile([1, CC], F32, tag="rec")
                    nc.vector.reciprocal(out=rec[0:1, 0:cc], in_=sp[0:1, 0:cc])
                    bp = mpsum.tile([NMIX, CC], F32, tag="bp")
                    nc.tensor.matmul(bp[0:NMIX, 0:cc], ones1_20[:], rec[0:1, 0:cc],
                                     start=True, stop=True)
                    nc.vector.tensor_tensor(out=o2[0:20, :], in0=epi[0:NMIX, 0:cc],
                                            in1=bp[0:NMIX, 0:cc], op=ALU.mult)
                    # mu copy (yp1[0:40])
                    nc.vector.tensor_copy(out=o1[0:40, :], in_=yp1[0:40, 0:cc])
                    # rho tanh (yp1[96:116])
                    nc.scalar.activation(out=o1[96:116, :], in_=yp1[96:116, 0:cc],
                                         func=AF.Tanh)
                    # eos sigmoid via tanh (yp1[64:65])
                    teos = mscr.tile([65, CC], F32, tag="teos")
                    nc.scalar.activation(out=teos[64:65, 0:cc], in_=yp1[64:65, 0:cc],
                                         func=AF.Tanh, scale=0.5)
                    nc.vector.tensor_scalar(out=o1[64:65, :], in0=teos[64:65, 0:cc],
                                            scalar1=0.5, scalar2=0.5,
                                            op0=ALU.mult, op1=ALU.add)
                    # s exp (yp2[32:72], split at quadrant boundary)
                    nc.scalar.activation(out=o2[32:64, :], in_=yp2[32:64, 0:cc],
                                         func=AF.Exp)
                    nc.scalar.activation(out=o2[64:72, :], in_=yp2[64:72, 0:cc],
                                         func=AF.Exp)
            nc.gpsimd.dma_start(out=dOUT1[:], in_=OUTS1[:])
            nc.gpsimd.dma_start(out=dOUT2[:], in_=OUTS2[:])
            if _DBG:
                nc.gpsimd.dma_start(out=dDH1, in_=h[0][:])
                nc.gpsimd.dma_start(out=dDH2, in_=h[1][:])
                nc.gpsimd.dma_start(out=dDH3, in_=h[2][:])
                nc.gpsimd.dma_start(out=dDC1, in_=czo[0][:])
                nc.gpsimd.dma_start(out=dDKAP, in_=kap[:])
                nc.gpsimd.dma_start(out=dDXW, in_=xwA[:])
                nc.gpsimd.dma_start(out=dDZ1, in_=dbgz[:])
                nc.gpsimd.dma_start(out=dDE, in_=dbgE[:])
                nc.gpsimd.dma_start(out=dDPHI, in_=dbgphi[:])

    nc.compile()
    return nc


def _prep_core(inputs, bsl, T):
    x = np.asarray(inputs['input_strokes'], np.float32)
    chars = np.asarray(inputs['input_chars'])
    lens = np.asarray(inputs['input_char_lens'])

    def W_of(l):
        if l == 0:
            Wx = np.asarray(inputs['Wx0'], np.float32)
            rows = [np.asarray(inputs['Wh0'], np.float32)]
        else:
            Wx = np.asarray(inputs['Wx%d' % l], np.float32)
            rows = [np.asarray(inputs['Wh%d' % l], np.float32), Wx[76:476]]
        b = np.asarray(inputs['b%d' % l], np.float32)
        nh = len(rows)
        nkt = 4 * nh + 1
        Wfull = np.zeros((128 * nkt, 1600), np.float32)
        for j, Whx in enumerate(rows):
            Wfull[512 * j: 512 * j + 400] = Whx
        base = 512 * nh                     # xw k-tile: w @0:73, x @96:99, b @99
        Wfull[base: base + 73] = Wx[3:76]
        Wfull[base + 96: base + 99] = Wx[0:3]
        Wfull[base + 99] = b
        Wfull[:, 1200:1600] *= 0.5          # o-gate prescale (sigmoid-via-tanh)
        return _pack_wblocks(Wfull, nkt)

    W1, W2, W3 = W_of(0), W_of(1), W_of(2)

    PB = np.zeros((128, 9 * 16), np.float32)
    for l in range(3):
        p = np.asarray(inputs['p%d' % l], np.float32)
        for j in range(3):
            pj = p[j] * (0.5 if j == 2 else 1.0)   # o-peephole prescale
            pbv = np.zeros((128, 16), np.float32)
            for blk in range(4):
                n = min(128, 400 - 128 * blk)
                pbv[0:n, 4 * blk: 4 * blk + 4] = pj[128 * blk: 128 * blk + n, None]
            PB[:, (3 * l + j) * 16: (3 * l + j) * 16 + 16] = pbv

    XT = np.zeros((3, (T + 1) * NB), np.float32)
    xs = x[bsl]
    for b in range(NB):
        XT[:, b:T * NB:NB] = xs[b].T
    WATT = np.zeros((128, 4 * 30), np.float32)
    wa = np.asarray(inputs['W_att'], np.float32)
    for k in range(4):
        n = min(128, 400 - 128 * k)
        WATT[0:n, 30 * k: 30 * k + 30] = wa[128 * k: 128 * k + n]
    WATTB = np.asarray(inputs['b_att'], np.float32).reshape(1, 30)
    V3 = np.concatenate([np.ones(UC), np.arange(UC),
                         np.arange(UC) ** 2]).astype(np.float32)[None, :]
    OHB = np.zeros((UC, NB * NCHARS), np.float32)
    for b, gb in enumerate(bsl):
        oh = np.zeros((UC, NCHARS), np.float32)
        oh[np.arange(UC), chars[gb].astype(int)] = 1.0
        oh[int(lens[gb]):] = 0.0
        OHB[:, NCHARS * b: NCHARS * b + NCHARS] = oh
    # WMDN: per k-tile block [m1(128) | m2(72)]; block 4 row 0 = biases
    wm = np.asarray(inputs['W_mdn'], np.float32)
    bm = np.asarray(inputs['b_mdn'], np.float32)
    wmf = np.zeros((513, 121), np.float32)
    wmf[0:400] = wm
    wmf[512] = bm                           # bias row, applied via ones moving
    m1 = np.zeros((513, 128), np.float32)
    m2 = np.zeros((513, 72), np.float32)
    m1[:, 0:40] = wmf[:, 20:60]             # mu1, mu2
    m1[:, 64:65] = wmf[:, 120:121]          # eos
    m1[:, 96:116] = wmf[:, 100:120]         # rho
    m2[:, 0:20] = wmf[:, 0:20]              # pi
    m2[:, 32:72] = wmf[:, 60:100]           # s1, s2
    WMDN = np.zeros((128, 5 * 200), np.float32)
    for k in range(4):
        WMDN[:, 200 * k: 200 * k + 128] = m1[128 * k: 128 * k + 128]
        WMDN[:, 200 * k + 128: 200 * k + 200] = m2[128 * k: 128 * k + 128]
    WMDN[0, 800:928] = m1[512]
    WMDN[0, 928:1000] = m2[512]
    XWI = np.zeros((XWROWS, NB), np.float32)
    XWI[96:99, :] = XT[:, 0:NB]             # x_0 prestaged (overwritten anyway)
    XWI[99, :] = 1.0                        # bias row
    return {'W1': W1, 'W2': W2, 'W3': W3, 'PB': PB,
            'XT': XT.astype(BF), 'WATT': WATT.astype(BF),
            'WATTB': WATTB.astype(BF), 'ONESB': np.ones((1, NB), BF),
            'V3': V3, 'OHB': OHB.astype(BF), 'WMDN': WMDN.astype(BF),
            'ONESC': np.ones((1, 400), BF),
            'XWI': XWI.astype(BF), 'ZB': np.zeros((128, 16), BF)}


def kernel(**inputs):
    x = np.asarray(inputs['input_strokes'])
    B, T, _ = x.shape
    if T not in _CACHE:
        _CACHE[T] = _build_program(T)
    nc = _CACHE[T]
    in_maps = [_prep_core(inputs, list(range(cr * NB, cr * NB + NB)), T)
               for cr in range(NCORES)]
    res = run_bass_kernel_spmd(nc, in_maps, list(range(NCORES)))
    outs = []
    for cr in range(NCORES):
        O1 = res.results[cr]['OUT1'].reshape(128, T, NB)
        O2 = res.results[cr]['OUT2'].reshape(72, T, NB)
        y = np.empty((NB, T, NOUT), np.float32)
        y[..., 0:20] = O2[0:20].transpose(2, 1, 0)
        y[..., 20:60] = O1[0:40].transpose(2, 1, 0)
        y[..., 60:100] = O2[32:72].transpose(2, 1, 0)
        y[..., 100:120] = O1[96:116].transpose(2, 1, 0)
        y[..., 120:121] = O1[64:65].transpose(2, 1, 0)
        outs.append(y)
    return np.concatenate(outs, 0).astype(np.float32)
